# revision 22
# baseline (speedup 1.0000x reference)
"""nn_GRUModel on 8 trn2 NeuronCores.

Strategy: 8-way tensor-parallel over the 3*H gate dimension.
  - Host: embedding gather + transpose to xeT [512, S*B] (s-major tokens),
    per-core weight slices (each core owns a 128-row slice of H, i.e. 384
    gate rows: r/z/n for its slice), bf16 casts.
  - Device per core:
      Phase 1: giT[384, S*B] = w_ih_part @ xeT + biases (kept in SBUF, bf16)
      Phase 2: 256 sequential GRU steps. Per step: 24 LDW+MM bf16 matmuls
        (w_hh chunk stationary, gathered hidden state streams), gate
        epilogue on ACT/DVE, partial FC matmul into PSUM -> DRAM, and an
        8-core AllGather of the new hidden chunk [128, 64].
      Phase 3: ReduceScatter of the partial logits, + fc_b, log_softmax,
        each core writes its S/8 slice of the output.
  - Host: assemble [B, S, D_OUT] from the 8 per-core slices.
"""
import os
import numpy as np
import ml_dtypes

VOCAB, D_IN, D_H, D_OUT, B, S = 32000, 512, 1024, 64, 64, 256
NCORES = 8
HP = D_H // NCORES          # 128 hidden rows per core
GP = 3 * HP                 # 384 gate rows per core
BF = np.dtype(ml_dtypes.bfloat16)

_S_RUN = int(os.environ.get("GRU_S", str(S)))
_EXCHANGE = os.environ.get("GRU_EXCHANGE", "cc")

_BUILD_CACHE = {}


def _build(s_run: int, exchange: str):
    key = (s_run, exchange)
    if key in _BUILD_CACHE:
        return _BUILD_CACHE[key]

    import concourse.bacc as bacc
    import concourse.tile as tile
    import concourse.mybir as mybir

    f32 = mybir.dt.float32
    bf16 = mybir.dt.bfloat16
    AF = mybir.ActivationFunctionType
    SB_TOK = s_run * B

    nc = bacc.Bacc(
        "TRN2", target_bir_lowering=False, debug=False, num_devices=NCORES
    )
    xeT_d = nc.dram_tensor("xeT", [D_IN, SB_TOK], bf16, kind="ExternalInput")
    wih_d = nc.dram_tensor("wih", [D_IN, GP], bf16, kind="ExternalInput")
    whh_d = nc.dram_tensor("whh", [D_H, GP], bf16, kind="ExternalInput")
    bgi_d = nc.dram_tensor("bgi", [HP, 3], f32, kind="ExternalInput")
    bhn_d = nc.dram_tensor("bhn", [1, HP], bf16, kind="ExternalInput")
    fcw_d = nc.dram_tensor("fcw", [HP, D_OUT], bf16, kind="ExternalInput")
    fcb_d = nc.dram_tensor("fcb", [128, D_OUT], f32, kind="ExternalInput")
    n_out_rows = SB_TOK // NCORES
    out_d = nc.dram_tensor(
        "out", [n_out_rows, D_OUT], f32, kind="ExternalOutput"
    )

    NTOKCH = SB_TOK // 512  # 512-token chunks for the projection

    with tile.TileContext(nc) as tc:
        with (
            tc.tile_pool(name="wpool", bufs=1) as wpool,
            tc.tile_pool(name="xpool", bufs=3) as xpool,
            tc.tile_pool(name="tpool", bufs=3) as tpool,
            tc.tile_pool(name="hpool", bufs=1) as hpool,
            tc.tile_pool(name="drpool", bufs=2, space="DRAM") as drpool,
            tc.tile_pool(name="dr1pool", bufs=1, space="DRAM") as dr1pool,
        ):
            # ---- resident tensors ----
            wih_sb = wpool.tile([128, 4, GP], bf16)
            whh_sb = wpool.tile([128, 8, GP], bf16)
            fcw_sb = wpool.tile([HP, D_OUT], bf16)
            bgi_sb = wpool.tile([HP, 3], f32)
            bhn_sb = wpool.tile([1, HP], bf16)
            fcb_sb = wpool.tile([128, D_OUT], f32)

            nc.sync.dma_start(wih_sb[:], wih_d.rearrange("(k p) g -> p k g", p=128))
            nc.sync.dma_start(whh_sb[:], whh_d.rearrange("(k p) g -> p k g", p=128))
            nc.sync.dma_start(fcw_sb[:], fcw_d[:])
            nc.sync.dma_start(bgi_sb[:], bgi_d[:])
            nc.sync.dma_start(bhn_sb[:], bhn_d[:])
            nc.sync.dma_start(fcb_sb[:], fcb_d[:])

            # ---- phases 1+2 fused: projection chunks are emitted into
            # the recurrence's collective windows (PE is idle during the
            # AllGather; projection chunk n covers steps 8n..8n+7) ----
            logitsP = dr1pool.tile([SB_TOK, D_OUT], f32)
            NCH = NTOKCH
            LOOKAHEAD = 3
            xeT_r = xeT_d.rearrange("(k p) t -> p k t", p=128)
            gi_tiles = []

            with (
                tc.tile_pool(
                    name="pj", bufs=2, space=bacc.bass.MemorySpace.PSUM
                ) as pjpool,
                tc.tile_pool(
                    name="gh", bufs=1, space=bacc.bass.MemorySpace.PSUM
                ) as ghpool,
                tc.tile_pool(
                    name="fcp", bufs=2, space=bacc.bass.MemorySpace.PSUM
                ) as fcpool,
                tc.tile_pool(name="hpool2", bufs=2) as hpool2,
            ):
                ones_row = wpool.tile([1, B], bf16)
                nc.gpsimd.memset(ones_row[:], 1.0)

                xe_tiles = {}

                def emit_chunk_dma(n):
                    xe_ch = xpool.tile([128, 4, 512], bf16, tag="xech",
                                       name=f"xech{n}")
                    nc.scalar.dma_start(
                        xe_ch[:], xeT_r[:, :, n * 512:(n + 1) * 512]
                    )
                    xe_tiles[n] = xe_ch

                def emit_chunk_mms(n):
                    xe_ch = xe_tiles.pop(n)
                    g = wpool.tile([128, 3, 512], bf16, name=f"gic{n}",
                                   tag=f"gic{n}")
                    for m in range(3):
                        ps = pjpool.tile([128, 512], f32, tag="pj",
                                         name=f"pj{n}_{m}")
                        for k in range(4):
                            nc.tensor.matmul(
                                ps[:],
                                wih_sb[:, k, m * 128:(m + 1) * 128],
                                xe_ch[:, k, :],
                                start=(k == 0),
                                stop=(k == 3),
                            )
                        nc.scalar.activation(
                            g[:, m, :], ps[:], AF.Identity,
                            bias=bgi_sb[:, m:m + 1],
                        )
                    gi_tiles.append(g)

                emit_chunk_dma(0)
                if NCH > 1:
                    emit_chunk_dma(1)
                emit_chunk_mms(0)

                ht8 = hpool2.tile([128, NCORES, B], bf16, tag="ht8",
                                  name="ht8_init")
                nc.gpsimd.memset(ht8[:], 0.0)
                h_a = hpool.tile([128, B], bf16)
                h_b = hpool.tile([128, B], bf16)
                nc.gpsimd.memset(h_a[:], 0.0)

                for t in range(s_run):
                    if t % 8 == 0:
                        if t // 8 + 2 < NCH:
                            emit_chunk_dma(t // 8 + 2)
                        if t // 8 + 1 < NCH:
                            emit_chunk_mms(t // 8 + 1)
                    h_prev = h_a if t % 2 == 0 else h_b
                    h_cur = h_b if t % 2 == 0 else h_a
                    psr = ghpool.tile([128, B], f32, tag="psr",
                                      name=f"psr{t}")
                    psz = ghpool.tile([128, B], f32, tag="psz",
                                      name=f"psz{t}")
                    psn = ghpool.tile([128, B], f32, tag="psn",
                                      name=f"psn{t}")
                    pss = (psr, psz, psn)
                    for m in range(3):
                        for k in range(NCORES):
                            nc.tensor.matmul(
                                pss[m][:],
                                whh_sb[:, k, m * 128:(m + 1) * 128],
                                ht8[:, k, :],
                                start=(k == 0),
                                stop=(k == NCORES - 1 and m != 2),
                            )
                    # fold b_hh_n into psn with a K=1 rank-1 matmul
                    nc.tensor.matmul(
                        psn[:], bhn_sb[:], ones_row[:],
                        start=False, stop=True,
                    )
                    g = gi_tiles[t // 8]
                    col = (t % 8) * B
                    gi_r = g[:, 0, col:col + B]
                    gi_z = g[:, 1, col:col + B]
                    gi_n = g[:, 2, col:col + B]

                    tr = tpool.tile([128, B], f32, tag="tr")
                    nc.vector.tensor_add(tr[:], psr[:], gi_r)
                    r = tpool.tile([128, B], f32, tag="r")
                    nc.scalar.activation(r[:], tr[:], AF.Sigmoid)

                    tz = tpool.tile([128, B], f32, tag="tz")
                    nc.vector.tensor_add(tz[:], psz[:], gi_z)
                    z = tpool.tile([128, B], f32, tag="z")
                    nc.scalar.activation(z[:], tz[:], AF.Sigmoid)

                    t1 = tpool.tile([128, B], f32, tag="t1")
                    nc.vector.tensor_mul(t1[:], r[:], psn[:])
                    t2 = tpool.tile([128, B], f32, tag="t2")
                    nc.vector.tensor_add(t2[:], t1[:], gi_n)
                    n_t = tpool.tile([128, B], f32, tag="n_t")
                    nc.scalar.activation(n_t[:], t2[:], AF.Tanh)

                    q_t = tpool.tile([128, B], f32, tag="q_t")
                    nc.vector.tensor_scalar(
                        q_t[:], z[:], -1.0, 1.0,
                        op0=mybir.AluOpType.mult,
                        op1=mybir.AluOpType.add,
                    )
                    p1 = tpool.tile([128, B], f32, tag="p1")
                    nc.vector.tensor_mul(p1[:], z[:], h_prev[:])
                    w1 = tpool.tile([128, B], f32, tag="w1")
                    nc.vector.tensor_mul(w1[:], n_t[:], q_t[:])
                    nc.vector.tensor_add(h_cur[:], w1[:], p1[:])

                    if t < s_run - 1:
                        cin = drpool.tile([128, B], bf16, tag="cin",
                                          name=f"cin{t}")
                        nc.sync.dma_start(cin[:], h_cur[:])
                        cout = drpool.tile(
                            [NCORES * 128, B], bf16,
                            addr_space="Shared", tag="cout",
                            name=f"cout{t}",
                        )
                        nc.gpsimd.collective_compute(
                            "AllGather",
                            mybir.AluOpType.bypass,
                            replica_groups=[list(range(NCORES))],
                            ins=[cin.opt()],
                            outs=[cout.opt()],
                        )
                        ht8 = hpool2.tile([128, NCORES, B], bf16,
                                          tag="ht8", name=f"ht8_{t}")
                        nc.sync.dma_start(
                            ht8[:],
                            cout.rearrange("(r p) f -> p r f", p=128),
                        )

                    # fc partial for this step (off the critical path)
                    psfc = fcpool.tile([B, D_OUT], f32, tag="fc",
                                       name=f"fc{t}")
                    nc.tensor.matmul(
                        psfc[:], h_cur[:], fcw_sb[:], start=True, stop=True
                    )
                    fct = tpool.tile([B, D_OUT], f32, tag="fct")
                    nc.vector.tensor_copy(fct[:], psfc[:])
                    nc.scalar.dma_start(
                        logitsP[t * B:(t + 1) * B, :], fct[:]
                    )

            # ---- phase 3: reduce-scatter logits, +fc_b, log_softmax ----
            rs_out = dr1pool.tile([n_out_rows, D_OUT], f32)
            nc.gpsimd.collective_compute(
                "ReduceScatter",
                mybir.AluOpType.add,
                replica_groups=[list(range(NCORES))],
                ins=[logitsP.opt()],
                outs=[rs_out.opt()],
            )
            n_ltiles = n_out_rows // 128
            for j in range(n_ltiles):
                lt = tpool.tile([128, D_OUT], f32, tag="lt")
                nc.sync.dma_start(lt[:], rs_out[j * 128:(j + 1) * 128, :])
                lb = tpool.tile([128, D_OUT], f32, tag="lb")
                nc.vector.tensor_add(lb[:], lt[:], fcb_sb[:])
                mx = tpool.tile([128, 1], f32, tag="mx")
                nc.vector.reduce_max(mx[:], lb[:], axis=mybir.AxisListType.X)
                nmx = tpool.tile([128, 1], f32, tag="nmx")
                nc.vector.tensor_scalar_mul(nmx[:], mx[:], -1.0)
                ex = tpool.tile([128, D_OUT], f32, tag="ex")
                sm = tpool.tile([128, 1], f32, tag="sm")
                nc.scalar.activation(
                    ex[:], lb[:], AF.Exp, bias=nmx[:], accum_out=sm[:]
                )
                ls_ = tpool.tile([128, 1], f32, tag="ls_")
                nc.scalar.activation(ls_[:], sm[:], AF.Ln)
                shift = tpool.tile([128, 1], f32, tag="shift")
                nc.vector.tensor_add(shift[:], mx[:], ls_[:])
                ot = tpool.tile([128, D_OUT], f32, tag="ot")
                nc.vector.tensor_scalar_sub(ot[:], lb[:], shift[:])
                nc.sync.dma_start(out_d[j * 128:(j + 1) * 128, :], ot[:])

    nc.compile()
    _BUILD_CACHE[key] = nc
    return nc


def _prep_inputs(x, emb, w_ih, w_hh, b_ih, b_hh, fc_w, fc_b, s_run):
    x = np.asarray(x).astype(np.int64)
    emb = np.asarray(emb, np.float32)
    w_ih = np.asarray(w_ih, np.float32)
    w_hh = np.asarray(w_hh, np.float32)
    b_ih = np.asarray(b_ih, np.float32)
    b_hh = np.asarray(b_hh, np.float32)
    fc_w = np.asarray(fc_w, np.float32)
    fc_b = np.asarray(fc_b, np.float32)

    xs = x[:, :s_run]
    xe = emb[xs]                                   # [B, s, 512]
    xeT = np.ascontiguousarray(
        xe.transpose(2, 1, 0).reshape(D_IN, s_run * B)
    ).astype(BF)

    bsum = b_ih + b_hh
    in_maps = []
    for i in range(NCORES):
        idx = np.concatenate(
            [np.arange(g * D_H + i * HP, g * D_H + i * HP + HP)
             for g in range(3)]
        )
        wih_i = np.ascontiguousarray(w_ih[idx].T).astype(BF)
        whh_i = np.ascontiguousarray(w_hh[idx].T).astype(BF)
        bgi_cols = []
        for m in range(3):
            src = bsum if m < 2 else b_ih
            bgi_cols.append(src[idx[m * HP:(m + 1) * HP]])
        bgi_i = np.stack(bgi_cols, axis=1).astype(np.float32)
        bhn_i = b_hh[2 * D_H + i * HP: 2 * D_H + i * HP + HP]
        bhn_i = bhn_i.reshape(1, HP).astype(BF)
        fcw_i = np.ascontiguousarray(fc_w[:, i * HP:(i + 1) * HP].T).astype(BF)
        fcb_t = np.ascontiguousarray(
            np.broadcast_to(fc_b, (128, D_OUT))
        ).astype(np.float32)
        in_maps.append({
            "xeT": xeT,
            "wih": wih_i,
            "whh": whh_i,
            "bgi": bgi_i,
            "bhn": bhn_i,
            "fcw": fcw_i,
            "fcb": fcb_t,
        })
    return in_maps


def run_on_device(x, emb, w_ih, w_hh, b_ih, b_hh, fc_w, fc_b,
                  s_run=None, exchange=None, trace=False):
    from concourse.bass_utils import run_bass_kernel_spmd

    s_run = _S_RUN if s_run is None else s_run
    exchange = _EXCHANGE if exchange is None else exchange
    nc = _build(s_run, exchange)
    in_maps = _prep_inputs(
        x, emb, w_ih, w_hh, b_ih, b_hh, fc_w, fc_b, s_run
    )
    spc = s_run // NCORES   # steps per core

    def _run_once():
        res = run_bass_kernel_spmd(
            nc, in_maps, list(range(NCORES)), trace=trace
        )
        out = np.empty((B, s_run, D_OUT), np.float32)
        for i in range(NCORES):
            oi = res.results[i]["out"].reshape(spc, B, D_OUT)
            out[:, i * spc:(i + 1) * spc, :] = oi.transpose(1, 0, 2)
        return out, res

    out, res = _run_once()
    if not np.isfinite(out).all():
        out, res = _run_once()
    return out, res


def kernel(x, emb, w_ih, w_hh, b_ih, b_hh, fc_w, fc_b):
    out, _ = run_on_device(x, emb, w_ih, w_hh, b_ih, b_hh, fc_w, fc_b)
    return out
